# revision 8
# baseline (speedup 1.0000x reference)
"""CondConv (per-sample routed 3x3 conv) on 8 Trainium2 NeuronCores — fp8 version.

Reference computation (all fp32):
    gap     = mean(x, axis=(2,3))                    [B, CIN]
    routing = sigmoid(gap @ W_att.T + b_att)         [B, E]
    ker     = einsum('be,eoihw->boihw', routing, convs)
    out[b]  = conv2d(x[b], ker[b], stride 1, pad 1)  [B, COUT, 56, 56]

Numerics: the conv runs on the PE in fp8e4 with MatmulPerfMode.DoubleRow
(contraction 256 = 2 cin-chunks per matmul, 2 fp8 weights/cell). Plain fp8
would lose ~5e-2 absmax because the mixed kernel has mean ~2; so the expert
bank is shifted host-side (C~_e = C_e - 0.5), making the mixed kernel small
(+-0.4), and the missing term  s_b * (ones3x3 (x) sum_cin x)  with
s_b = 0.5 * sum_e r_be  is added back exactly via a bf16 box-filter path:
PE ones-matmul column sums -> separable 3-tap W/H adds on DVE/GpSimd ->
fused into the PSUM drains (STT: out = box*s_b + psum). Validated ~1e-2
absmax, vs the 2e-2 gate.

Sharding (B=32, COUT=256 across 8 cores): 4 core-pairs; pair p owns samples
8p..8p+7, each core of a pair computes one half of COUT (128 channels).

Layouts: x planes are zero-padded to 58x58 (+12 tail, so the k-pair chunk
stride 3376 is %16 for DoubleRow); every conv rhs is one flat contiguous
464 window (8 rows); garbage output columns 56,57 of each row are never
drained. The kernel mix runs in bf16 as 8 tensor_scalar_mul (4x DVE mode)
+ a 7-add tree (2 adds DVE 2x mode, 5 GpSimd TensorTensor — walrus allows
only TensorTensor/copy on GpSimd, and GpSimd cannot access PSUM), with the
final GpSimd add writing the fp8 kernel directly.
"""

import numpy as np
import ml_dtypes

B, CIN, H, W = 32, 256, 56, 56
COUT, KK, E = 256, 3, 8
HP, WP = H + 2, H + 2          # padded plane rows x row-stride
PW = HP * WP                   # 3364
PWA = PW + 12                  # pad to %16 bytes; flat windows never overrun
NSH = KK * KK                  # 9 shifts
CHUNKS = 2                     # CIN = 2 * 128
MHALF = COUT // 2              # couts per core
RPT = 8                        # output rows per psum tile
NT = H // RPT                  # 7
NF = RPT * WP                  # 464 flat free per psum tile
NCORES = 8
S = B // (NCORES // 2)         # 8 samples per core
MIXF = CHUNKS * NSH * MHALF    # 2304 mix free size

_cached = {}


def _build_program():
    import concourse.bacc as bacc
    import concourse.bass_isa as bass_isa
    import concourse.mybir as mybir
    from concourse.tile import TileContext

    f32 = mybir.dt.float32
    bf16 = mybir.dt.bfloat16
    f8 = mybir.dt.float8e4
    Alu = mybir.AluOpType
    Act = mybir.ActivationFunctionType
    PM = mybir.MatmulPerfMode
    AX = mybir.AxisListType

    nc = bacc.Bacc(None, target_bir_lowering=False)

    xb_d = nc.declare_dram_parameter("xb", [S, 128, CHUNKS, PWA], bf16, isOutput=False)
    cvb_d = nc.declare_dram_parameter("cvb", [E, 128, MIXF], bf16, isOutput=False)
    watt_d = nc.declare_dram_parameter("watt", [CHUNKS, 128, E], f32, isOutput=False)
    battb_d = nc.declare_dram_parameter("battb", [128, E], f32, isOutput=False)
    out_d = nc.declare_dram_parameter("out", [S, MHALF, H, W], bf16, isOutput=True)

    with TileContext(nc) as tc:
        with (
            tc.tile_pool(name="resident", bufs=1) as res_pool,
            tc.tile_pool(name="xb", bufs=3) as xb_pool,
            tc.tile_pool(name="x8", bufs=3) as x8_pool,
            tc.tile_pool(name="cs", bufs=1) as cs_pool,
            tc.tile_pool(name="wt", bufs=1) as wt_pool,
            tc.tile_pool(name="bx", bufs=2) as bx_pool,
            tc.tile_pool(name="mx", bufs=1) as mx_pool,
            tc.tile_pool(name="kt", bufs=2) as kt_pool,
            tc.tile_pool(name="small", bufs=3) as small_pool,
            tc.tile_pool(name="outsb", bufs=6) as out_pool,
            tc.tile_pool(name="cpsum", bufs=1, space="PSUM") as cps_pool,
            tc.tile_pool(name="cspsum", bufs=1, space="PSUM") as csp_pool,
        ):
            # ---- resident tiles ------------------------------------------
            watt_sb = []
            for c in range(CHUNKS):
                t = res_pool.tile([128, E], f32, name=f"watt{c}", tag=f"watt{c}")
                nc.sync.dma_start(out=t[:], in_=watt_d[c])
                watt_sb.append(t)
            battb_sb = res_pool.tile([128, E], f32, name="battb", tag="battb")
            nc.sync.dma_start(out=battb_sb[:], in_=battb_d[:])
            # colsum lhsT: 0.5 folds the s_b = 0.5*sum_e r_be factor into box
            ones_sb = res_pool.tile([128, 128], bf16, name="ones", tag="ones")
            nc.vector.memset(ones_sb[:], 0.5)
            scal_sb = res_pool.tile([128, S * E], f32, name="scal", tag="scal")
            sbv_sb = res_pool.tile([128, S], f32, name="sbv", tag="sbv")

            cv_sb = [None] * E

            # sample-0 bank DMA / mix run in 3-shift thirds so conv(0)
            # starts after only a third of the bank has landed
            THIRDS = [slice(g * 3 * CHUNKS * MHALF, (g + 1) * 3 * CHUNKS * MHALF)
                      for g in range(3)]
            NCS = NT + 1               # colsum tiles per sample

            def emit_bank():
                for e in range(E):
                    cv_sb[e] = res_pool.tile([128, MIXF], bf16,
                                             name=f"cv{e}", tag=f"cv{e}")
                for sl in THIRDS:
                    for e in range(E):
                        nc.sync.dma_start(out=cv_sb[e][:, sl], in_=cvb_d[e, :, sl])

            def emit_loadx(b):
                t = xb_pool.tile([128, CHUNKS, PWA], bf16, name="xb", tag="xb")
                half = PWA // 2
                for c in range(CHUNKS):
                    for j in range(2):
                        sl = slice(j * half, (j + 1) * half)
                        nc.sync.dma_start(out=t[:, c, sl], in_=xb_d[b, :, c, sl])
                return t

            def emit_gap_cast(b, xb):
                """ScalarE: bf16 -> fp8 cast of the padded planes; the
                accumulator of each chunk's pass is the GAP row-sum."""
                x8 = x8_pool.tile([128, CHUNKS, PWA], f8, name="x8", tag="x8")
                gq = []
                for c in range(CHUNKS):
                    g = small_pool.tile([128, 1], f32, name=f"g{c}", tag=f"g{c}")
                    nc.scalar.activation(out=x8[:, c, :], in_=xb[:, c, :],
                                         func=Act.Copy, accum_out=g[:])
                    gq.append(g)
                return x8, gq

            def emit_routing(b, gq):
                t0 = small_pool.tile([128, E], f32, name="t0", tag="t0")
                nc.vector.tensor_scalar_mul(out=t0[:], in0=watt_sb[0][:],
                                            scalar1=gq[0][:, 0:1])
                t1 = small_pool.tile([128, E], f32, name="t1", tag="t1")
                nc.vector.scalar_tensor_tensor(
                    out=t1[:], in0=watt_sb[1][:], scalar=gq[1][:, 0:1],
                    in1=t0[:], op0=Alu.mult, op1=Alu.add)
                red = small_pool.tile([128, E], f32, name="red", tag="red")
                nc.gpsimd.partition_all_reduce(red[:], t1[:], channels=128,
                                               reduce_op=bass_isa.ReduceOp.add)
                red2 = small_pool.tile([128, E], f32, name="red2", tag="red2")
                nc.vector.tensor_add(out=red2[:], in0=red[:], in1=battb_sb[:])
                nc.scalar.activation(out=scal_sb[:, b * E:(b + 1) * E],
                                     in_=red2[:], func=Act.Sigmoid)
                nc.vector.tensor_reduce(out=sbv_sb[:, b:b + 1],
                                        in_=scal_sb[:, b * E:(b + 1) * E],
                                        axis=AX.X, op=Alu.add)

            def emit_mix(b, split=False):
                """ker~(b) = sum_e r_be * C~_e in bf16: 8 muls (DVE 4x mode)
                + 7-add tree (2 adds DVE 2x mode, 5 GpSimd; the final GpSimd
                add writes the fp8 kernel directly, cast fused). With
                split=True the whole pipeline runs per shift-half so conv(0)
                s-groups 0..4 start before the bank DMA fully lands."""
                kt = kt_pool.tile([128, NSH, CHUNKS, MHALF], f8, name="kt", tag="kt")
                ktf = kt.rearrange("p s c m -> p (s c m)")
                m = [mx_pool.tile([128, MIXF], bf16, name=f"m{e}", tag=f"m{e}")
                     for e in range(E)]
                sls = THIRDS if split else [slice(0, MIXF)]
                sc = lambda e: scal_sb[:, b * E + e:b * E + e + 1]
                for sl in sls:
                    # walrus only allows TensorTensor/copy on GpSimd, so all
                    # 8 muls ride DVE (4x mode); add tree 2 DVE + 5 GpSimd
                    for e in range(E):
                        nc.vector.tensor_scalar_mul(
                            out=m[e][:, sl], in0=cv_sb[e][:, sl], scalar1=sc(e))
                    nc.vector.tensor_add(out=m[0][:, sl], in0=m[0][:, sl],
                                         in1=m[1][:, sl])
                    nc.vector.tensor_add(out=m[2][:, sl], in0=m[2][:, sl],
                                         in1=m[3][:, sl])
                    nc.gpsimd.tensor_add(out=m[4][:, sl], in0=m[4][:, sl],
                                         in1=m[5][:, sl])
                    nc.gpsimd.tensor_add(out=m[6][:, sl], in0=m[6][:, sl],
                                         in1=m[7][:, sl])
                    nc.gpsimd.tensor_add(out=m[0][:, sl], in0=m[0][:, sl],
                                         in1=m[2][:, sl])
                    nc.gpsimd.tensor_add(out=m[4][:, sl], in0=m[4][:, sl],
                                         in1=m[6][:, sl])
                    nc.gpsimd.tensor_add(out=ktf[:, sl], in0=m[0][:, sl],
                                         in1=m[4][:, sl])
                return kt

            def emit_colsum_pair(b, xb, cs, t):
                """One box column-sum tile: 2 bf16 ones-matmuls (both cin
                chunks) into the spare PSUM bank + ScalarE drain to SBUF."""
                if t < NT:
                    off, sz = t * NF, NF
                else:
                    off, sz = NT * NF, PWA - NT * NF  # rows 56,57 + tail
                p = csp_pool.tile([128, sz], f32, name=f"csp{t}", tag="csp")
                nc.tensor.matmul(p[:], ones_sb[:], xb[:, 0, off:off + sz],
                                 start=True, stop=False)
                nc.tensor.matmul(p[:], ones_sb[:], xb[:, 1, off:off + sz],
                                 start=False, stop=True)
                nc.scalar.activation(out=cs[:, off:off + sz], in_=p[:],
                                     func=Act.Copy)

            def emit_taps(b, cs):
                """Separable 3x3 ones filter on the 0.5-scaled column sums:
                W-tap then H-tap, bf16 tensor adds split DVE/GpSimd, packed
                to valid width (W-tap 58x56, H-tap 56x56)."""
                c3 = cs[:, 0:PW].rearrange("p (r q) -> p r q", q=WP)
                wt = wt_pool.tile([128, HP, W], bf16, name="wt", tag="wt")
                nc.vector.tensor_add(out=wt[:], in0=c3[:, 0:HP, 0:W],
                                     in1=c3[:, 0:HP, 1:W + 1])
                nc.gpsimd.tensor_add(out=wt[:], in0=wt[:],
                                     in1=c3[:, 0:HP, 2:W + 2])
                wf = wt.rearrange("p r q -> p (r q)")
                bx = bx_pool.tile([128, H * W], bf16, name="bx", tag="bx")
                nc.vector.tensor_add(out=bx[:], in0=wf[:, 0:H * W],
                                     in1=wf[:, W:(H + 1) * W])
                nc.gpsimd.tensor_add(out=bx[:], in0=bx[:],
                                     in1=wf[:, 2 * W:(H + 2) * W])
                return bx

            def emit_conv(b, x8, kt, bx, csjobs):
                """Conv for sample b: 63 fp8 DoubleRow matmuls (9 shifts x 7
                tiles, contraction 256), with the NEXT sample's box column-sum
                pairs interleaved into the PE stream; drains fuse this
                sample's box term via STT on DVE."""
                cps = [cps_pool.tile([128, NF], f32, name=f"cps{n}", tag=f"cps{n}")
                       for n in range(NT)]

                def emit_drain(n, bx):
                    # GPSIMD cannot touch PSUM on real HW: all drains on DVE
                    o = out_pool.tile([128, RPT, W], bf16, name="osb", tag="osb")
                    p3 = cps[n].rearrange("p (r q) -> p r q", q=WP)
                    b3 = bx.rearrange("p (r q) -> p r q", q=W)
                    nc.vector.scalar_tensor_tensor(
                        out=o[:], in0=b3[:, n * RPT:(n + 1) * RPT, :],
                        scalar=sbv_sb[:, b:b + 1], in1=p3[:, :, 0:W],
                        op0=Alu.mult, op1=Alu.add)
                    nc.sync.dma_start(
                        out=out_d[b, :, n * RPT:(n + 1) * RPT, :], in_=o[:])

                def mm(s, n):
                    dh, dw = s // KK, s % KK
                    off = (n * RPT + dh) * WP + dw
                    nc.tensor.matmul(cps[n][:], kt[:, s, :, :],
                                     x8[:, :, off:off + NF],
                                     start=(s == 0), stop=(s == NSH - 1),
                                     perf_mode=PM.DoubleRow)

                # shifts-outer: 7 matmuls share each lhsT; one colsum pair
                # of the next sample rides ahead of each shift group. The
                # last two shift groups run tile-major so each tile's stop
                # fires early and the serialized DVE drains get a head start.
                for s in range(NSH - 2):
                    if s < len(csjobs):
                        csjobs[s]()
                    for n in range(NT):
                        mm(s, n)
                for j in csjobs[NSH - 2:]:
                    j()
                for n in range(NT):
                    mm(NSH - 2, n)
                    mm(NSH - 1, n)
                for n in range(NT):
                    emit_drain(n, bx)

            # ---- software-pipelined emission -----------------------------
            # Prologue: colsum(0) runs standalone on the otherwise-idle PE
            # while the bank DMA + split mix(0) pipeline fills; every conv(b)
            # then interleaves colsum(b+1) pairs into its matmul stream.
            def new_cs():
                return cs_pool.tile([128, PWA], bf16, name="cs", tag="cs")

            xbs = {0: emit_loadx(0)}
            emit_bank()
            xbs[1] = emit_loadx(1)
            x8s = {}
            x8s[0], gq0 = emit_gap_cast(0, xbs[0])
            emit_routing(0, gq0)
            kts = {0: emit_mix(0, split=True)}
            css = {0: new_cs()}
            for t in range(NCS):
                emit_colsum_pair(0, xbs[0], css[0], t)
            bxs = {0: emit_taps(0, css[0])}
            xbs[2] = emit_loadx(2)
            x8s[1], gq1 = emit_gap_cast(1, xbs[1])
            emit_routing(1, gq1)

            for b in range(S):
                if b + 1 < S:
                    kts[b + 1] = emit_mix(b + 1)
                jobs = []
                if b + 1 < S:
                    css[b + 1] = new_cs()
                    jobs = [
                        lambda t=t, s=b + 1: emit_colsum_pair(s, xbs[s], css[s], t)
                        for t in range(NCS)
                    ]
                emit_conv(b, x8s[b], kts[b], bxs[b], jobs)
                if b + 1 < S:
                    bxs[b + 1] = emit_taps(b + 1, css[b + 1])
                if b + 3 < S:
                    xbs[b + 3] = emit_loadx(b + 3)
                if b + 2 < S:
                    x8s[b + 2], gq = emit_gap_cast(b + 2, xbs[b + 2])
                    emit_routing(b + 2, gq)
                xbs.pop(b, None)
                kts.pop(b, None)
                x8s.pop(b - 1, None)
                css.pop(b, None)
                bxs.pop(b - 1, None)

    nc.compile()
    return nc


def _prep_core_inputs(x, convs, W_att, b_att):
    """Host-side shard/layout prep. Returns list of 8 per-core input dicts."""
    f32 = np.float32
    bf16 = ml_dtypes.bfloat16

    # padded bf16 planes [B, 128, 2, PWA]
    xr = np.ascontiguousarray(x, dtype=f32).reshape(B, CHUNKS, 128, H, W)
    xpad = np.zeros((B, CHUNKS, 128, HP, WP), dtype=bf16)
    xpad[:, :, :, 1:H + 1, 1:W + 1] = xr.astype(bf16)
    xpad = xpad.reshape(B, CHUNKS, 128, PW)
    xbf = np.zeros((B, 128, CHUNKS, PWA), dtype=bf16)
    xbf[:, :, :, :PW] = xpad.transpose(0, 2, 1, 3)

    # shifted expert bank per cout-half: [E, p, (kh kw), c, m] bf16
    cv = np.ascontiguousarray(convs, dtype=f32) - f32(0.5)
    cvh = cv.reshape(E, 2, MHALF, CHUNKS, 128, KK, KK)
    cvb_halves = [
        np.ascontiguousarray(
            cvh[:, h].transpose(0, 3, 4, 5, 2, 1).reshape(E, 128, MIXF)
        ).astype(bf16)
        for h in range(2)
    ]

    watt = np.ascontiguousarray(
        (np.asarray(W_att, dtype=f32).T / f32(H * W)).reshape(CHUNKS, 128, E))
    battb = np.ascontiguousarray(
        np.broadcast_to(np.asarray(b_att, dtype=f32), (128, E)))

    in_maps = []
    for k in range(NCORES):
        pair, half = k // 2, k % 2
        sl = slice(pair * S, (pair + 1) * S)
        in_maps.append({
            "xb": xbf[sl],
            "cvb": cvb_halves[half],
            "watt": watt,
            "battb": battb,
        })
    return in_maps


def _assemble_output(results):
    out = np.empty((B, COUT, H, W), dtype=np.float32)
    for k in range(NCORES):
        pair, half = k // 2, k % 2
        sl = slice(pair * S, (pair + 1) * S)
        out[sl, half * MHALF:(half + 1) * MHALF] = np.asarray(
            results[k]["out"]).astype(np.float32)
    return out


def kernel(x, convs, W_att, b_att):
    from concourse.bass_utils import run_bass_kernel_spmd

    if "nc" not in _cached:
        _cached["nc"] = _build_program()
    in_maps = _prep_core_inputs(x, convs, W_att, b_att)
    res = run_bass_kernel_spmd(_cached["nc"], in_maps, core_ids=list(range(NCORES)))
    return _assemble_output(res.results)
